# revision 12
# baseline (speedup 1.0000x reference)
"""Trainium2 Bass kernel for CoExDispProcessor (topk_masking), v2.

Per-sample computation (data-parallel over batch across 8 cores):
  1. top-2 over the D=48 disparity axis of cost [1,48,128,240] -> softmax
     blend of the two indices -> disp4 [128,240]
  2. 3x3 unfold of disp4 (zero pad) -> nearest 4x upsample -> weighted sum
     with softmax over the 9 channels of spg [9,512,960] -> disp1 [512,960]

v2 design (replaces the per-column MAX8/FIND_INDEX8 approach that burned
~144us of DVE):
  - cost is quantized on ACT to a u16 hi-plane q16 = round(v*4096+32768) and
    an ACT+GpSimd lo-plane r16 = (v*4096+32768-q16)*64512+32768 (residual of
    the rounding, so hi/lo are self-consistent by construction).  Top-2 values
    and argmax indices come from fp16-rate stt max-trees + equality/iota
    passes on DVE (all u16, contiguous, eligible for the 2x/4x DVE modes).
  - the two-candidate blend uses Sigmoid on ACT directly: disp4 =
    i1 + (i2-i1)*sigmoid((q2-q1)/4096).
  - fine stage: exp on ACT (fp16 out), 9-channel weighted num/den
    accumulation with scalar_tensor_tensor (4x-eligible), den partially on
    GpSimd, reciprocal on ACT (no Newton), fp16 output tensor.
  - DMA: cost split across sync+tensor queues (960B lines), spg streamed on
    the tensor queue, outputs on sync.
"""

import os
import sys
from contextlib import ExitStack

import numpy as np

if "/opt/trn_rl_repo" not in sys.path:
    sys.path.insert(0, "/opt/trn_rl_repo")

import concourse.bass as bass
import concourse.bacc as bacc
import concourse.tile as tile
from concourse import mybir
from concourse.bass_utils import run_bass_kernel_spmd

F32 = mybir.dt.float32
FP16 = mybir.dt.float16
U16 = mybir.dt.uint16
OP = mybir.AluOpType
ACT = mybir.ActivationFunctionType

B, D, H, W = 8, 48, 128, 240
HF, WF = 4 * H, 4 * W  # 512, 960
N_CORES = 8

HALF0 = 121            # coarse cols [0,121) -> covers everything fine-k0 needs
S_R = 64512.0          # lo-plane scale (keeps r16 strictly inside (0, 65535))
H_SCALE = 4096.0 * S_R
H_BIAS = 32768.0 * S_R + 32768.0
NDCH = 6               # cost d-chunks of 8
KW = WF // 2           # 480 fine cols per k chunk
DEN_GPS = 288          # fine cols of each chunk accumulated on gpsimd (c1..c6)


def _act_reciprocal(nc, out_ap, in_ap):
    eng = nc.scalar
    return eng.add_instruction(
        mybir.InstActivation(
            name=nc.get_next_instruction_name(),
            func=ACT.Reciprocal,
            ins=[
                eng.lower_ap(in_ap),
                mybir.ImmediateValue(dtype=F32, value=0.0),
                mybir.ImmediateValue(dtype=F32, value=1.0),
                mybir.ImmediateValue(dtype=F32, value=0.0),
            ],
            outs=[eng.lower_ap(out_ap)],
        )
    )


def build_kernel(ctx: ExitStack, tc: tile.TileContext, out_d, cost_d, spg_d):
    nc = tc.nc
    stt = nc.vector.scalar_tensor_tensor
    ts = nc.vector.tensor_scalar

    cost_hdw = cost_d.transpose([1, 0, 2])  # [128(h), 48(d), 240(w)] view
    spg_v = spg_d.rearrange("c (R dr) (k w) -> c R dr k w", dr=4, k=2)
    out_v = out_d.rearrange("(R dr) (k w) -> R dr k w", dr=4, k=2)

    # ---- persistent tiles --------------------------------------------------
    pers = ctx.enter_context(tc.tile_pool(name="pers", bufs=1))
    q16 = pers.tile([128, D, W], U16)
    r16 = pers.tile([128, D, W], U16)
    iota1 = pers.tile([128, D, HALF0], U16)  # value d+1, widest half
    rv = []
    urep = []
    for s in range(3):
        rv_s = pers.tile([128, W + 2], F32, tag=f"rv{s}")
        rv.append(rv_s)
        urep_s = pers.tile([128, 4 * (W + 2)], FP16, tag=f"urep{s}")
        urep.append(urep_s)

    small = ctx.enter_context(tc.tile_pool(name="small", bufs=1))

    for s in range(3):
        nc.vector.memset(rv[s][:], 0.0)
    # iota over d (+1), built by doubling from a gpsimd iota of 8
    nc.gpsimd.iota(iota1[:, 0:8, :], [[1, 8], [0, HALF0]], base=1,
                   channel_multiplier=0)
    ts(iota1[:, 8:16, :], iota1[:, 0:8, :], 8, None, op0=OP.add)
    ts(iota1[:, 16:32, :], iota1[:, 0:16, :], 16, None, op0=OP.add)
    ts(iota1[:, 32:48, :], iota1[:, 16:32, :], 16, None, op0=OP.add)

    # ---- cost DMA + packing into q16/r16 ----------------------------------
    with ExitStack() as phase1:
        cpool = phase1.enter_context(tc.tile_pool(name="costc", bufs=3))
        hpool = phase1.enter_context(tc.tile_pool(name="hres", bufs=2))
        ctiles = []
        for ci in range(NDCH):
            t = cpool.tile([128, 8, W], F32, tag="c")
            eng = nc.sync if ci < 3 else nc.scalar
            eng.dma_start(t[:], cost_hdw[:, ci * 8:(ci + 1) * 8, :])
            ctiles.append(t)
        for ci in range(NDCH):
            dsl = slice(ci * 8, (ci + 1) * 8)
            nc.scalar.activation(q16[:, dsl, :], ctiles[ci][:], ACT.Copy,
                                 bias=32768.0, scale=4096.0)
            hres = hpool.tile([128, 8, W], F32, tag="h")
            nc.scalar.activation(hres[:], ctiles[ci][:], ACT.Copy,
                                 bias=H_BIAS, scale=H_SCALE)
            stt(r16[:, dsl, :], q16[:, dsl, :], -S_R, hres[:],
                op0=OP.mult, op1=OP.add)

    # ---- spg stream (scalar queue) + exp tiles ----------------------------
    raw_pool = ctx.enter_context(tc.tile_pool(name="raw", bufs=3))
    e_pool = ctx.enter_context(tc.tile_pool(name="epool", bufs=8))
    fin = ctx.enter_context(tc.tile_pool(name="fin", bufs=1))
    p_pool = ctx.enter_context(tc.tile_pool(name="ppool", bufs=2))
    out_pool = ctx.enter_context(tc.tile_pool(name="outp", bufs=2))
    work = ctx.enter_context(tc.tile_pool(name="work", bufs=2))
    tp = ctx.enter_context(tc.tile_pool(name="trees", bufs=1))

    SPG_ORDER = [(k, c) for k in range(2) for c in range(9)]
    raws = {}
    spg_issued = [0]

    def issue_spg():
        if spg_issued[0] >= len(SPG_ORDER):
            return
        k, c = SPG_ORDER[spg_issued[0]]
        spg_issued[0] += 1
        raw = raw_pool.tile([128, 4, KW], F32, tag="raw")
        nc.scalar.dma_start(raw[:], spg_v[c, :, :, k, :])
        raws[(k, c)] = raw

    for _ in range(3):  # prefill the raw pool
        issue_spg()

    e_tiles = {}
    for k in range(2):
        for c in range(9):
            e_kc = e_pool.tile([128, 4, KW], FP16, tag="e")
            e_tiles[(k, c)] = e_kc

    def emit_exp(k, c):
        nc.scalar.activation(e_tiles[(k, c)][:], raws[(k, c)][:], ACT.Exp)
        issue_spg()

    # ---- top-2 machinery ---------------------------------------------------
    def wtile(nw):
        t = work.tile([128, D, HALF0], U16, tag="w")
        return t[:, :, 0:nw]

    def umax_tree(src, dst, nw):
        """src: [128,48,nw] u16 AP view; dst: [128,nw] u16 tile (via view)."""
        cur, n = src, D
        lvl = 0
        while n > 3:
            h = n // 2
            t = tp.tile([128, h, HALF0], U16, tag=f"lv{lvl}")
            tv = t[:, :, 0:nw]
            stt(tv, cur[:, 0:h, :], 0, cur[:, h:n, :], op0=OP.max, op1=OP.max)
            cur, n = tv, h
            lvl += 1
        t1 = tp.tile([128, 1, HALF0], U16, tag="lvf")
        t1v = t1[:, :, 0:nw]
        stt(t1v, cur[:, 0:1, :], 0, cur[:, 1:2, :], op0=OP.max, op1=OP.max)
        stt(dst[:].unsqueeze(1), t1v, 0, cur[:, 2:3, :], op0=OP.max, op1=OP.max)

    def topk_half(a, b):
        nw = b - a
        q = q16[:, :, a:b]
        iw = iota1[:, :, 0:nw]
        bshape = [128, D, nw]

        q1 = small.tile([128, nw], U16, tag=f"q1_{a}")
        umax_tree(q, q1, nw)
        eq1 = wtile(nw)
        stt(eq1, q, 0, q1[:].unsqueeze(1).broadcast_to(bshape),
            op0=OP.add, op1=OP.is_equal)
        ieq1 = wtile(nw)
        stt(ieq1, eq1, 1, iw, op0=OP.mult, op1=OP.mult)
        i1p = small.tile([128, nw], U16, tag=f"i1_{a}")
        umax_tree(ieq1, i1p, nw)
        mnot = wtile(nw)
        stt(mnot, ieq1, 0, i1p[:].unsqueeze(1).broadcast_to(bshape),
            op0=OP.add, op1=OP.not_equal)
        # masked back into q16 (in place): q16 * (position != argmax)
        stt(q, mnot, 1, q, op0=OP.mult, op1=OP.mult)
        q2 = small.tile([128, nw], U16, tag=f"q2_{a}")
        umax_tree(q, q2, nw)
        eq2 = wtile(nw)
        stt(eq2, q, 0, q2[:].unsqueeze(1).broadcast_to(bshape),
            op0=OP.add, op1=OP.is_equal)
        r2 = wtile(nw)
        stt(r2, eq2, 1, r16[:, :, a:b], op0=OP.mult, op1=OP.mult)
        lo2 = small.tile([128, nw], U16, tag=f"lo2_{a}")
        umax_tree(r2, lo2, nw)
        e3 = wtile(nw)
        stt(e3, r2, 0, lo2[:].unsqueeze(1).broadcast_to(bshape),
            op0=OP.add, op1=OP.is_equal)
        ieq3 = wtile(nw)
        stt(ieq3, e3, 1, iw, op0=OP.mult, op1=OP.mult)
        i2p = small.tile([128, nw], U16, tag=f"i2_{a}")
        umax_tree(ieq3, i2p, nw)

        # smalls: disp4 = i1 + (i2-i1)*sigmoid((q2-q1)/4096)
        i1f = small.tile([128, nw], F32, tag=f"i1f_{a}")
        ts(i1f[:], i1p[:], 1.0, None, op0=OP.subtract)
        i2f = small.tile([128, nw], F32, tag=f"i2f_{a}")
        ts(i2f[:], i2p[:], 1.0, None, op0=OP.subtract)
        q1f = small.tile([128, nw], F32, tag=f"q1f_{a}")
        nc.vector.tensor_copy(q1f[:], q1[:])
        q2f = small.tile([128, nw], F32, tag=f"q2f_{a}")
        nc.vector.tensor_copy(q2f[:], q2[:])
        dq = small.tile([128, nw], F32, tag=f"dq_{a}")
        nc.vector.tensor_sub(dq[:], q2f[:], q1f[:])
        sig = small.tile([128, nw], F32, tag=f"sig_{a}")
        nc.scalar.activation(sig[:], dq[:], ACT.Sigmoid, scale=1.0 / 4096.0)
        di = small.tile([128, nw], F32, tag=f"di_{a}")
        nc.vector.tensor_sub(di[:], i2f[:], i1f[:])
        tm = small.tile([128, nw], F32, tag=f"tm_{a}")
        nc.vector.tensor_mul(tm[:], di[:], sig[:])
        nc.vector.tensor_add(rv[1][:, 1 + a:1 + b], tm[:], i1f[:])

    def rv_urep_part(a, b):
        # shift rows via SBUF->SBUF DMA, then x4 col-repeat into fp16 urep
        ra, rb = a + 1, b + 1
        if a == 0:
            ra = 0
        if b == W:
            rb = W + 2
        nc.sync.dma_start(rv[0][1:128, ra:rb], rv[1][0:127, ra:rb])
        nc.sync.dma_start(rv[2][0:127, ra:rb], rv[1][1:128, ra:rb])
        for s in range(3):
            nc.scalar.copy(
                urep[s][:, 4 * ra:4 * rb].rearrange("p (x dw) -> p x dw", dw=4),
                rv[s][:, ra:rb].unsqueeze(2).broadcast_to([128, rb - ra, 4]),
            )

    # ---- fine stage --------------------------------------------------------
    num_t, den_t, r0_t = {}, {}, {}
    for k in range(2):
        num_k = fin.tile([128, 4, KW], FP16, tag=f"num{k}")
        num_t[k] = num_k
        den_k = fin.tile([128, 4, KW], FP16, tag=f"den{k}")
        den_t[k] = den_k
        r0_k = fin.tile([128, 4, KW], FP16, tag=f"r0{k}")
        r0_t[k] = r0_k

    def u4(k, c):
        ci, cj = c // 3, c % 3
        off = 4 * (k * (W // 2) + cj)
        return (urep[ci][:, off:off + KW]
                .unsqueeze(1).broadcast_to([128, 4, KW]))

    def fine_mul(k, c):
        e = e_tiles[(k, c)]
        num = num_t[k]
        if c == 0:
            stt(num[:], e[:], 1.0, u4(k, 0), op0=OP.mult, op1=OP.mult)
        else:
            p = p_pool.tile([128, 4, KW], FP16, tag="p")
            stt(p[:], e[:], 1.0, u4(k, c), op0=OP.mult, op1=OP.mult)
            stt(num[:], p[:], 0, num[:], op0=OP.add, op1=OP.add)

    def fine_den(k, c):
        # den accumulation: channels 1..6 on gpsimd, c7/c8 on DVE (tail)
        den = den_t[k]
        e = e_tiles[(k, c)]
        if c == 1:
            nc.gpsimd.tensor_add(den[:], e_tiles[(k, 0)][:], e[:])
        elif c <= 6:
            nc.gpsimd.tensor_add(den[:], den[:], e[:])
        else:
            stt(den[:], e[:], 0, den[:], op0=OP.add, op1=OP.add)

    def fine_final(k):
        r0 = r0_t[k]
        _act_reciprocal(nc, r0[:], den_t[k][:])
        outt = out_pool.tile([128, 4, KW], FP16, tag="outt")
        stt(outt[:], num_t[k][:], 4.0, r0[:], op0=OP.mult, op1=OP.mult)
        nc.sync.dma_start(out_v[:, :, k, :], outt[:])

    # ---- schedule ----------------------------------------------------------
    for c in range(9):
        emit_exp(0, c)

    topk_half(0, HALF0)
    rv_urep_part(0, HALF0)

    for c in range(9):         # fine k0 compute (needs urep part 0 only)
        fine_mul(0, c)
        if c >= 1:
            fine_den(0, c)

    topk_half(HALF0, W)
    rv_urep_part(HALF0, W)
    fine_final(0)

    for c in range(9):
        emit_exp(1, c)
        fine_mul(1, c)
        if c >= 1:
            fine_den(1, c)
    fine_final(1)


def build_program():
    nc = bacc.Bacc(
        "TRN2",
        target_bir_lowering=False,
        debug=False,
        enable_asserts=False,
        num_devices=N_CORES,
    )
    cost_d = nc.dram_tensor("cost", [D, H, W], F32, kind="ExternalInput").ap()
    spg_d = nc.dram_tensor("spg", [9, HF, WF], F32, kind="ExternalInput").ap()
    out_d = nc.dram_tensor("out", [HF, WF], FP16, kind="ExternalOutput").ap()
    with tile.TileContext(nc) as tc:
        with ExitStack() as ctx:
            build_kernel(ctx, tc, out_d, cost_d, spg_d)
    nc.compile()
    return nc


def _install_ntff_hook():
    """Provide antenv.axon_hooks + register the ctypes NTFF profiler."""
    import types

    if "antenv.axon_hooks" in sys.modules:
        return True
    try:
        import antenv
        from trn_agent_boot.trn_boot import _ntff_profile_via_ctypes

        mod = types.ModuleType("antenv.axon_hooks")
        mod._hook = None

        def set_axon_ntff_profile_hook(hook):
            mod._hook = hook

        def get_axon_ntff_profile_hook():
            return mod._hook

        mod.set_axon_ntff_profile_hook = set_axon_ntff_profile_hook
        mod.get_axon_ntff_profile_hook = get_axon_ntff_profile_hook
        sys.modules["antenv.axon_hooks"] = mod
        antenv.axon_hooks = mod
        mod._hook = _ntff_profile_via_ctypes("/opt/axon/libaxon_pjrt.so")
        return True
    except Exception as e:  # profiling is best-effort
        print(f"NTFF hook install failed: {e}")
        return False


LAST_RESULTS = None


def kernel(cost: np.ndarray, spg: np.ndarray) -> np.ndarray:
    """cost [8,1,48,128,240] f32, spg [8,9,512,960] f32 -> disp1 [8,512,960] f32."""
    global LAST_RESULTS
    cost = np.ascontiguousarray(np.asarray(cost, dtype=np.float32))
    spg = np.ascontiguousarray(np.asarray(spg, dtype=np.float32))
    assert cost.shape == (B, 1, D, H, W) and spg.shape == (B, 9, HF, WF)

    nc = build_program()
    in_maps = [
        {"cost": cost[b, 0], "spg": spg[b]} for b in range(B)
    ]
    trace = bool(int(os.environ.get("KERNEL_TRACE", "0")))
    if trace:
        trace = _install_ntff_hook()
    res = run_bass_kernel_spmd(
        nc, in_maps, core_ids=list(range(N_CORES)), trace=trace
    )
    LAST_RESULTS = res
    out = np.stack([np.asarray(res.results[b]["out"]) for b in range(B)], axis=0)
    return out.astype(np.float32)


# revision 29
# speedup vs baseline: 1.6863x; 1.6863x over previous
"""Trainium2 Bass kernel for CoExDispProcessor (topk_masking), v5 hybrid.

Per-sample computation (data-parallel over batch across 8 cores):
  1. top-2 over the D=48 disparity axis of cost [1,48,128,240] -> softmax
     blend of the two indices -> disp4 [128,240]
  2. 3x3 unfold of disp4 (zero pad) -> nearest 4x upsample -> weighted sum
     with softmax over the 9 channels of spg [9,512,960] -> disp1 [512,960]

Design (informed by measured TRN2 rates):
  - top-2 values via the native MAX8 instruction per w column (exact f32
    top-8, no index instructions).  disp4 is then computed index-free as a
    masked softmax-weighted index sum:
        M = (cost >= m2),  eta = exp(cost)
        disp4 = sum_d d*eta*M / sum_d eta*M
    which equals the reference top-2 blend exactly (m2 = second largest).
  - the x4 disp scale is folded into the urep copy (ACT, scale=4).
  - fine stage: exp on ACT (fp16), per-channel tensor_mul with broadcast
    urep slices, num accumulation via tensor_add, den fully on gpsimd
    (contiguous full-channel adds), ACT reciprocal (no Newton), fp16 out.
"""

import os
import sys
from contextlib import ExitStack

import numpy as np

if "/opt/trn_rl_repo" not in sys.path:
    sys.path.insert(0, "/opt/trn_rl_repo")

import concourse.bass as bass
import concourse.bacc as bacc
import concourse.tile as tile
from concourse import mybir
from concourse.bass_utils import run_bass_kernel_spmd

F32 = mybir.dt.float32
FP16 = mybir.dt.float16
OP = mybir.AluOpType
ACT = mybir.ActivationFunctionType

B, D, H, W = 8, 48, 128, 240
HF, WF = 4 * H, 4 * W  # 512, 960
N_CORES = 8

COST_CHUNKS = [48, 48, 48, 48, 48]  # w columns per cost DMA chunk
KW = WF // 2                        # 480 fine cols per k chunk


def _act_reciprocal(nc, out_ap, in_ap):
    eng = nc.scalar
    return eng.add_instruction(
        mybir.InstActivation(
            name=nc.get_next_instruction_name(),
            func=ACT.Reciprocal,
            ins=[
                eng.lower_ap(in_ap),
                mybir.ImmediateValue(dtype=F32, value=0.0),
                mybir.ImmediateValue(dtype=F32, value=1.0),
                mybir.ImmediateValue(dtype=F32, value=0.0),
            ],
            outs=[eng.lower_ap(out_ap)],
        )
    )


def build_kernel(ctx: ExitStack, tc: tile.TileContext, out_d, cost_d, spg_d):
    nc = tc.nc
    tt = nc.vector.tensor_tensor

    cost_hdw = cost_d.transpose([1, 0, 2])  # [128(h), 48(d), 240(w)] view
    spg_v = spg_d.rearrange("c (R dr) (k w) -> c R dr k w", dr=4, k=2)
    out_v = out_d.rearrange("(R dr) (k w) -> R dr k w", dr=4, k=2)

    # ---- persistent tiles --------------------------------------------------
    pers = ctx.enter_context(tc.tile_pool(name="pers", bufs=1))
    rv = []
    urep = []
    for s_ in range(3):
        rv_s = pers.tile([128, W + 2], F32, tag=f"rv{s_}")
        rv.append(rv_s)
        urep_s = pers.tile([128, 4 * (W + 2)], FP16, tag=f"urep{s_}")
        urep.append(urep_s)
    small = ctx.enter_context(tc.tile_pool(name="small", bufs=1))
    for s_ in range(3):
        nc.vector.memset(rv[s_][:], 0.0)

    # ---- program-lifetime fine pools (stack allocator: first = outermost) --
    raw_pool = ctx.enter_context(tc.tile_pool(name="raw", bufs=2))
    e_pool = ctx.enter_context(tc.tile_pool(name="epool", bufs=9))
    fin = ctx.enter_context(tc.tile_pool(name="fin", bufs=1))
    r0p = ctx.enter_context(tc.tile_pool(name="r0p", bufs=1))
    p_pool = ctx.enter_context(tc.tile_pool(name="ppool", bufs=2))
    out_pool = ctx.enter_context(tc.tile_pool(name="outp", bufs=1))

    SPG_ORDER = [(k, c) for k in range(2) for c in range(9)]
    raws = {}
    spg_issued = [0]

    def issue_spg():
        if spg_issued[0] >= len(SPG_ORDER):
            return
        k, c = SPG_ORDER[spg_issued[0]]
        spg_issued[0] += 1
        raw = raw_pool.tile([128, 4, KW], F32, tag="raw")
        nc.scalar.dma_start(raw[:], spg_v[c, :, :, k, :])
        raws[(k, c)] = raw

    for _ in range(2):
        issue_spg()

    e_tiles = {}
    for k in range(2):
        for c in range(9):
            e_kc = e_pool.tile([128, 4, KW], FP16, tag="e")
            e_tiles[(k, c)] = e_kc

    def emit_exp(k, c):
        nc.scalar.activation(e_tiles[(k, c)][:], raws[(k, c)][:], ACT.Exp)
        issue_spg()

    # ---- topk scope --------------------------------------------------------
    topk_scope = ExitStack()
    tkp = topk_scope.enter_context(tc.tile_pool(name="tkp", bufs=1))
    mtp = topk_scope.enter_context(tc.tile_pool(name="mtp", bufs=1))
    lsp = topk_scope.enter_context(tc.tile_pool(name="lsp", bufs=1))

    costF = tkp.tile([128, D, W], F32)
    v8 = tkp.tile([128, W, 8], F32)
    eta = tkp.tile([128, D, W], FP16)
    iotaF = tkp.tile([128, D], FP16)
    nc.gpsimd.iota(iotaF[:], [[1, D]], base=0, channel_multiplier=0,
                   allow_small_or_imprecise_dtypes=True)

    # cost DMA by w chunks (sync queue) + per-column MAX8 as chunks land
    w0 = 0
    wchunks = []
    for nw in COST_CHUNKS:
        nc.sync.dma_start(costF[:, :, w0:w0 + nw], cost_hdw[:, :, w0:w0 + nw])
        wchunks.append((w0, nw))
        w0 += nw
    # eta = exp(cost), one full-width ACT pass, then the spg exps for k0
    nc.scalar.activation(eta[:], costF[:], ACT.Exp)
    for c in range(9):
        emit_exp(0, c)

    for w0, nw in wchunks:
        for j in range(nw):
            nc.vector.max(out=v8[:, w0 + j], in_=costF[:, :, w0 + j])

    m2c = small.tile([128, W], F32, tag="m2c")
    nc.vector.tensor_copy(m2c[:], v8[:, :, 1])
    bshape = [128, D, W]
    M = mtp.tile([128, D, W], FP16, tag="mt")
    tt(M[:], costF[:], m2c[:].unsqueeze(1).broadcast_to(bshape), op=OP.is_ge)
    tt(eta[:], eta[:], M[:], op=OP.mult)  # eta *= M

    def sum_ladder(dst, src3d):
        t = lsp.tile([128, 24, W], FP16, tag="ls")
        tt(t[:], src3d[:, 0:24, :], src3d[:, 24:48, :], op=OP.add)
        tt(t[:, 0:12, :], t[:, 0:12, :], t[:, 12:24, :], op=OP.add)
        tt(t[:, 0:6, :], t[:, 0:6, :], t[:, 6:12, :], op=OP.add)
        tt(t[:, 0:3, :], t[:, 0:3, :], t[:, 3:6, :], op=OP.add)
        tt(t[:, 0:1, :], t[:, 0:1, :], t[:, 1:2, :], op=OP.add)
        tt(dst[:].unsqueeze(1), t[:, 0:1, :], t[:, 2:3, :], op=OP.add)

    den2 = small.tile([128, W], F32, tag="den2")
    sum_ladder(den2, eta[:])
    tt(eta[:], eta[:], iotaF[:].unsqueeze(2).broadcast_to(bshape), op=OP.mult)
    num2 = small.tile([128, W], F32, tag="num2")
    sum_ladder(num2, eta[:])
    rd2 = small.tile([128, W], F32, tag="rd2")
    nc.vector.reciprocal(rd2[:], den2[:])
    nc.vector.tensor_mul(rv[1][:, 1:1 + W], num2[:], rd2[:])

    topk_scope.close()

    # ---- rv row shifts + x4 col repeat (x4 disp scale folded in) ----------
    nc.sync.dma_start(rv[0][1:128, 1:W + 2], rv[1][0:127, 1:W + 2])
    nc.sync.dma_start(rv[2][0:127, 1:W + 2], rv[1][1:128, 1:W + 2])
    for s_ in range(3):
        nc.scalar.activation(
            urep[s_][:].rearrange("p (x dw) -> p x dw", dw=4),
            rv[s_][:].unsqueeze(2).broadcast_to([128, W + 2, 4]),
            ACT.Copy, scale=4.0)

    # ---- fine stage --------------------------------------------------------
    num_t, den_t = {}, {}
    for k in range(2):
        num_k = fin.tile([128, 4, KW], FP16, tag=f"num{k}")
        num_t[k] = num_k
        den_k = fin.tile([128, 4, KW], FP16, tag=f"den{k}")
        den_t[k] = den_k

    def u4(k, c):
        ci, cj = c // 3, c % 3
        off = 4 * (k * (W // 2) + cj)
        return (urep[ci][:, off:off + KW]
                .unsqueeze(1).broadcast_to([128, 4, KW]))

    def fine_channel(k, c):
        e = e_tiles[(k, c)]
        num = num_t[k]
        den = den_t[k]
        if c == 0:
            nc.vector.tensor_mul(num[:], e[:], u4(k, 0))
        else:
            p = p_pool.tile([128, 4, KW], FP16, tag="p")
            nc.vector.tensor_mul(p[:], e[:], u4(k, c))
            nc.vector.tensor_add(num[:], num[:], p[:])
        # den fully on gpsimd (contiguous full-channel adds)
        if c == 1:
            nc.gpsimd.tensor_add(den[:], e_tiles[(k, 0)][:], e[:])
        elif c >= 2:
            nc.gpsimd.tensor_add(den[:], den[:], e[:])

    def fine_final(k):
        r0 = r0p.tile([128, 4, KW], FP16, tag="r0")
        _act_reciprocal(nc, r0[:], den_t[k][:])
        outt = out_pool.tile([128, 4, KW], FP16, tag="outt")
        nc.vector.tensor_mul(outt[:], num_t[k][:], r0[:])
        nc.sync.dma_start(out_v[:, :, k, :], outt[:])

    for c in range(9):
        fine_channel(0, c)
    fine_final(0)
    for c in range(9):
        emit_exp(1, c)
        fine_channel(1, c)
    fine_final(1)


def build_program():
    nc = bacc.Bacc(
        "TRN2",
        target_bir_lowering=False,
        debug=False,
        enable_asserts=False,
        num_devices=N_CORES,
    )
    cost_d = nc.dram_tensor("cost", [D, H, W], F32, kind="ExternalInput").ap()
    spg_d = nc.dram_tensor("spg", [9, HF, WF], F32, kind="ExternalInput").ap()
    out_d = nc.dram_tensor("out", [HF, WF], FP16, kind="ExternalOutput").ap()
    with tile.TileContext(nc) as tc:
        with ExitStack() as ctx:
            build_kernel(ctx, tc, out_d, cost_d, spg_d)
    nc.compile()
    return nc


def _install_ntff_hook():
    """Provide antenv.axon_hooks + register the ctypes NTFF profiler."""
    import types

    if "antenv.axon_hooks" in sys.modules:
        return True
    try:
        import antenv
        from trn_agent_boot.trn_boot import _ntff_profile_via_ctypes

        mod = types.ModuleType("antenv.axon_hooks")
        mod._hook = None

        def set_axon_ntff_profile_hook(hook):
            mod._hook = hook

        def get_axon_ntff_profile_hook():
            return mod._hook

        mod.set_axon_ntff_profile_hook = set_axon_ntff_profile_hook
        mod.get_axon_ntff_profile_hook = get_axon_ntff_profile_hook
        sys.modules["antenv.axon_hooks"] = mod
        antenv.axon_hooks = mod
        mod._hook = _ntff_profile_via_ctypes("/opt/axon/libaxon_pjrt.so")
        return True
    except Exception as e:  # profiling is best-effort
        print(f"NTFF hook install failed: {e}")
        return False


LAST_RESULTS = None


def kernel(cost: np.ndarray, spg: np.ndarray) -> np.ndarray:
    """cost [8,1,48,128,240] f32, spg [8,9,512,960] f32 -> disp1 [8,512,960] f32."""
    global LAST_RESULTS
    cost = np.ascontiguousarray(np.asarray(cost, dtype=np.float32))
    spg = np.ascontiguousarray(np.asarray(spg, dtype=np.float32))
    assert cost.shape == (B, 1, D, H, W) and spg.shape == (B, 9, HF, WF)

    nc = build_program()
    in_maps = [
        {"cost": cost[b, 0], "spg": spg[b]} for b in range(B)
    ]
    trace = bool(int(os.environ.get("KERNEL_TRACE", "0")))
    if trace:
        trace = _install_ntff_hook()
    res = run_bass_kernel_spmd(
        nc, in_maps, core_ids=list(range(N_CORES)), trace=trace
    )
    LAST_RESULTS = res
    out = np.stack([np.asarray(res.results[b]["out"]) for b in range(B)], axis=0)
    return out.astype(np.float32)


# revision 32
# speedup vs baseline: 1.7098x; 1.0140x over previous
"""Trainium2 Bass kernel for CoExDispProcessor (topk_masking), v5 hybrid.

Per-sample computation (data-parallel over batch across 8 cores):
  1. top-2 over the D=48 disparity axis of cost [1,48,128,240] -> softmax
     blend of the two indices -> disp4 [128,240]
  2. 3x3 unfold of disp4 (zero pad) -> nearest 4x upsample -> weighted sum
     with softmax over the 9 channels of spg [9,512,960] -> disp1 [512,960]

Design (informed by measured TRN2 rates):
  - top-2 values via the native MAX8 instruction per w column (exact f32
    top-8, no index instructions).  disp4 is then computed index-free as a
    masked softmax-weighted index sum:
        M = (cost >= m2),  eta = exp(cost)
        disp4 = sum_d d*eta*M / sum_d eta*M
    which equals the reference top-2 blend exactly (m2 = second largest).
  - the x4 disp scale is folded into the urep copy (ACT, scale=4).
  - fine stage: exp on ACT (fp16), per-channel tensor_mul with broadcast
    urep slices, num accumulation via tensor_add, den fully on gpsimd
    (contiguous full-channel adds), ACT reciprocal (no Newton), fp16 out.
"""

import os
import sys
from contextlib import ExitStack

import numpy as np

if "/opt/trn_rl_repo" not in sys.path:
    sys.path.insert(0, "/opt/trn_rl_repo")

import concourse.bass as bass
import concourse.bacc as bacc
import concourse.tile as tile
from concourse import mybir
from concourse.bass_utils import run_bass_kernel_spmd

F32 = mybir.dt.float32
FP16 = mybir.dt.float16
OP = mybir.AluOpType
ACT = mybir.ActivationFunctionType

B, D, H, W = 8, 48, 128, 240
HF, WF = 4 * H, 4 * W  # 512, 960
N_CORES = 8

COST_CHUNKS = [24, 54, 54, 54, 54]  # w columns per cost DMA chunk
KW = WF // 2                        # 480 fine cols per k chunk


def _act_reciprocal(nc, out_ap, in_ap):
    eng = nc.scalar
    return eng.add_instruction(
        mybir.InstActivation(
            name=nc.get_next_instruction_name(),
            func=ACT.Reciprocal,
            ins=[
                eng.lower_ap(in_ap),
                mybir.ImmediateValue(dtype=F32, value=0.0),
                mybir.ImmediateValue(dtype=F32, value=1.0),
                mybir.ImmediateValue(dtype=F32, value=0.0),
            ],
            outs=[eng.lower_ap(out_ap)],
        )
    )


def build_kernel(ctx: ExitStack, tc: tile.TileContext, out_d, cost_d, spg_d):
    nc = tc.nc
    tt = nc.vector.tensor_tensor

    cost_hdw = cost_d.transpose([1, 0, 2])  # [128(h), 48(d), 240(w)] view
    spg_v = spg_d.rearrange("c (R dr) (k w) -> c R dr k w", dr=4, k=2)
    out_v = out_d.rearrange("(R dr) (k w) -> R dr k w", dr=4, k=2)

    # ---- persistent tiles --------------------------------------------------
    pers = ctx.enter_context(tc.tile_pool(name="pers", bufs=1))
    rv = []
    urep = []
    for s_ in range(3):
        rv_s = pers.tile([128, W + 2], F32, tag=f"rv{s_}")
        rv.append(rv_s)
        urep_s = pers.tile([128, 4 * (W + 2)], FP16, tag=f"urep{s_}")
        urep.append(urep_s)
    small = ctx.enter_context(tc.tile_pool(name="small", bufs=1))
    for s_ in range(3):
        nc.vector.memset(rv[s_][:], 0.0)

    # ---- program-lifetime fine pools (stack allocator: first = outermost) --
    raw_pool = ctx.enter_context(tc.tile_pool(name="raw", bufs=2))
    e_pool = ctx.enter_context(tc.tile_pool(name="epool", bufs=9))
    fin = ctx.enter_context(tc.tile_pool(name="fin", bufs=1))
    r0p = ctx.enter_context(tc.tile_pool(name="r0p", bufs=1))
    p_pool = ctx.enter_context(tc.tile_pool(name="ppool", bufs=1))
    out_pool = ctx.enter_context(tc.tile_pool(name="outp", bufs=1))

    SPG_ORDER = [(k, c) for k in range(2) for c in range(9)]
    raws = {}
    spg_issued = [0]

    def issue_spg():
        if spg_issued[0] >= len(SPG_ORDER):
            return
        k, c = SPG_ORDER[spg_issued[0]]
        spg_issued[0] += 1
        raw = raw_pool.tile([128, 4, KW], F32, tag="raw")
        nc.scalar.dma_start(raw[:], spg_v[c, :, :, k, :])
        raws[(k, c)] = raw

    for _ in range(2):
        issue_spg()

    e_tiles = {}
    for k in range(2):
        for c in range(9):
            e_kc = e_pool.tile([128, 4, KW], FP16, tag="e")
            e_tiles[(k, c)] = e_kc

    def emit_exp(k, c):
        nc.scalar.activation(e_tiles[(k, c)][:], raws[(k, c)][:], ACT.Exp)
        issue_spg()

    # ---- topk scope --------------------------------------------------------
    topk_scope = ExitStack()
    tkp = topk_scope.enter_context(tc.tile_pool(name="tkp", bufs=1))
    mtp = topk_scope.enter_context(tc.tile_pool(name="mtp", bufs=2))

    costF = tkp.tile([128, D, W], F32)
    v8 = tkp.tile([128, W, 8], F32)
    eta = tkp.tile([128, D, W], FP16)
    iotaF = tkp.tile([128, D], FP16)
    nc.gpsimd.iota(iotaF[:], [[1, D]], base=0, channel_multiplier=0,
                   allow_small_or_imprecise_dtypes=True)

    # cost DMA by w chunks (sync queue) + per-column MAX8 as chunks land
    w0 = 0
    wchunks = []
    for nw in COST_CHUNKS:
        nc.sync.dma_start(costF[:, :, w0:w0 + nw], cost_hdw[:, :, w0:w0 + nw])
        wchunks.append((w0, nw))
        w0 += nw
    # eta = exp(cost), one full-width ACT pass, then the spg exps for k0
    nc.scalar.activation(eta[:], costF[:], ACT.Exp)
    for c in range(9):
        emit_exp(0, c)

    for w0, nw in wchunks:
        for j in range(nw):
            nc.vector.max(out=v8[:, w0 + j], in_=costF[:, :, w0 + j])

    m2c = small.tile([128, W], F32, tag="m2c")
    nc.vector.tensor_copy(m2c[:], v8[:, :, 1])
    bshape = [128, D, W]
    M = mtp.tile([128, D, W], FP16, tag="mt")
    tt(M[:], costF[:], m2c[:].unsqueeze(1).broadcast_to(bshape), op=OP.is_ge)
    etaM = mtp.tile([128, D, W], FP16, tag="mt")
    tt(etaM[:], eta[:], M[:], op=OP.mult)

    def sum_ladder(dst, src3d):
        t_full = mtp.tile([128, D, W], FP16, tag="mt")
        t = t_full[:, 0:24, :]
        tt(t[:], src3d[:, 0:24, :], src3d[:, 24:48, :], op=OP.add)
        tt(t[:, 0:12, :], t[:, 0:12, :], t[:, 12:24, :], op=OP.add)
        tt(t[:, 0:6, :], t[:, 0:6, :], t[:, 6:12, :], op=OP.add)
        tt(t[:, 0:3, :], t[:, 0:3, :], t[:, 3:6, :], op=OP.add)
        tt(t[:, 0:1, :], t[:, 0:1, :], t[:, 1:2, :], op=OP.add)
        tt(dst[:].unsqueeze(1), t[:, 0:1, :], t[:, 2:3, :], op=OP.add)

    den2 = small.tile([128, W], F32, tag="den2")
    sum_ladder(den2, etaM[:])
    tt(eta[:], etaM[:], iotaF[:].unsqueeze(2).broadcast_to(bshape), op=OP.mult)
    num2 = small.tile([128, W], F32, tag="num2")
    sum_ladder(num2, eta[:])
    nc.vector.reciprocal(den2[:], den2[:])
    nc.vector.tensor_mul(rv[1][:, 1:1 + W], num2[:], den2[:])

    topk_scope.close()

    # ---- rv row shifts + x4 col repeat (x4 disp scale folded in) ----------
    nc.sync.dma_start(rv[0][1:128, 1:W + 2], rv[1][0:127, 1:W + 2])
    nc.sync.dma_start(rv[2][0:127, 1:W + 2], rv[1][1:128, 1:W + 2])
    for s_ in range(3):
        nc.scalar.activation(
            urep[s_][:].rearrange("p (x dw) -> p x dw", dw=4),
            rv[s_][:].unsqueeze(2).broadcast_to([128, W + 2, 4]),
            ACT.Copy, scale=4.0)

    # ---- fine stage --------------------------------------------------------
    num_t, den_t = {}, {}
    for k in range(2):
        num_k = fin.tile([128, 4, KW], FP16, tag=f"num{k}")
        num_t[k] = num_k
        den_k = fin.tile([128, 4, KW], FP16, tag=f"den{k}")
        den_t[k] = den_k

    def u4(k, c):
        ci, cj = c // 3, c % 3
        off = 4 * (k * (W // 2) + cj)
        return (urep[ci][:, off:off + KW]
                .unsqueeze(1).broadcast_to([128, 4, KW]))

    def fine_channel(k, c):
        e = e_tiles[(k, c)]
        num = num_t[k]
        den = den_t[k]
        if c == 0:
            nc.vector.tensor_mul(num[:], e[:], u4(k, 0))
        else:
            p = p_pool.tile([128, 4, KW], FP16, tag="p")
            nc.vector.tensor_mul(p[:], e[:], u4(k, c))
            nc.vector.tensor_add(num[:], num[:], p[:])
        # den: c1..c4 on gpsimd (contiguous adds), c5..c8 on DVE (tail)
        if c == 1:
            nc.gpsimd.tensor_add(den[:], e_tiles[(k, 0)][:], e[:])
        elif 2 <= c <= 4:
            nc.gpsimd.tensor_add(den[:], den[:], e[:])
        elif c >= 5:
            nc.vector.tensor_add(den[:], den[:], e[:])

    def fine_final(k):
        r0 = r0p.tile([128, 4, KW], FP16, tag="r0")
        _act_reciprocal(nc, r0[:], den_t[k][:])
        outt = out_pool.tile([128, 4, KW], FP16, tag="outt")
        nc.vector.tensor_mul(outt[:], num_t[k][:], r0[:])
        nc.sync.dma_start(out_v[:, :, k, :], outt[:])

    for c in range(9):
        fine_channel(0, c)
    fine_final(0)
    for c in range(9):
        emit_exp(1, c)
        fine_channel(1, c)
    fine_final(1)


def build_program():
    nc = bacc.Bacc(
        "TRN2",
        target_bir_lowering=False,
        debug=False,
        enable_asserts=False,
        num_devices=N_CORES,
    )
    cost_d = nc.dram_tensor("cost", [D, H, W], F32, kind="ExternalInput").ap()
    spg_d = nc.dram_tensor("spg", [9, HF, WF], F32, kind="ExternalInput").ap()
    out_d = nc.dram_tensor("out", [HF, WF], FP16, kind="ExternalOutput").ap()
    with tile.TileContext(nc) as tc:
        with ExitStack() as ctx:
            build_kernel(ctx, tc, out_d, cost_d, spg_d)
    nc.compile()
    return nc


def _install_ntff_hook():
    """Provide antenv.axon_hooks + register the ctypes NTFF profiler."""
    import types

    if "antenv.axon_hooks" in sys.modules:
        return True
    try:
        import antenv
        from trn_agent_boot.trn_boot import _ntff_profile_via_ctypes

        mod = types.ModuleType("antenv.axon_hooks")
        mod._hook = None

        def set_axon_ntff_profile_hook(hook):
            mod._hook = hook

        def get_axon_ntff_profile_hook():
            return mod._hook

        mod.set_axon_ntff_profile_hook = set_axon_ntff_profile_hook
        mod.get_axon_ntff_profile_hook = get_axon_ntff_profile_hook
        sys.modules["antenv.axon_hooks"] = mod
        antenv.axon_hooks = mod
        mod._hook = _ntff_profile_via_ctypes("/opt/axon/libaxon_pjrt.so")
        return True
    except Exception as e:  # profiling is best-effort
        print(f"NTFF hook install failed: {e}")
        return False


LAST_RESULTS = None


def kernel(cost: np.ndarray, spg: np.ndarray) -> np.ndarray:
    """cost [8,1,48,128,240] f32, spg [8,9,512,960] f32 -> disp1 [8,512,960] f32."""
    global LAST_RESULTS
    cost = np.ascontiguousarray(np.asarray(cost, dtype=np.float32))
    spg = np.ascontiguousarray(np.asarray(spg, dtype=np.float32))
    assert cost.shape == (B, 1, D, H, W) and spg.shape == (B, 9, HF, WF)

    nc = build_program()
    in_maps = [
        {"cost": cost[b, 0], "spg": spg[b]} for b in range(B)
    ]
    trace = bool(int(os.environ.get("KERNEL_TRACE", "0")))
    if trace:
        trace = _install_ntff_hook()
    res = run_bass_kernel_spmd(
        nc, in_maps, core_ids=list(range(N_CORES)), trace=trace
    )
    LAST_RESULTS = res
    out = np.stack([np.asarray(res.results[b]["out"]) for b in range(B)], axis=0)
    return out.astype(np.float32)


# revision 33
# speedup vs baseline: 1.8042x; 1.0552x over previous
"""Trainium2 Bass kernel for CoExDispProcessor (topk_masking), v5 hybrid.

Per-sample computation (data-parallel over batch across 8 cores):
  1. top-2 over the D=48 disparity axis of cost [1,48,128,240] -> softmax
     blend of the two indices -> disp4 [128,240]
  2. 3x3 unfold of disp4 (zero pad) -> nearest 4x upsample -> weighted sum
     with softmax over the 9 channels of spg [9,512,960] -> disp1 [512,960]

Design (informed by measured TRN2 rates):
  - top-2 values via the native MAX8 instruction per w column (exact f32
    top-8, no index instructions).  disp4 is then computed index-free as a
    masked softmax-weighted index sum:
        M = (cost >= m2),  eta = exp(cost)
        disp4 = sum_d d*eta*M / sum_d eta*M
    which equals the reference top-2 blend exactly (m2 = second largest).
  - the x4 disp scale is folded into the urep copy (ACT, scale=4).
  - fine stage: exp on ACT (fp16), per-channel tensor_mul with broadcast
    urep slices, num accumulation via tensor_add, den fully on gpsimd
    (contiguous full-channel adds), ACT reciprocal (no Newton), fp16 out.
"""

import os
import sys
from contextlib import ExitStack

import numpy as np

if "/opt/trn_rl_repo" not in sys.path:
    sys.path.insert(0, "/opt/trn_rl_repo")

import concourse.bass as bass
import concourse.bacc as bacc
import concourse.tile as tile
from concourse import mybir
from concourse.bass_utils import run_bass_kernel_spmd

F32 = mybir.dt.float32
FP16 = mybir.dt.float16
OP = mybir.AluOpType
ACT = mybir.ActivationFunctionType

B, D, H, W = 8, 48, 128, 240
HF, WF = 4 * H, 4 * W  # 512, 960
N_CORES = 8

COST_CHUNKS = [24, 54, 54, 54, 54]  # w columns per cost DMA chunk
KW = WF // 2                        # 480 fine cols per k chunk


def _act_reciprocal(nc, out_ap, in_ap):
    eng = nc.scalar
    return eng.add_instruction(
        mybir.InstActivation(
            name=nc.get_next_instruction_name(),
            func=ACT.Reciprocal,
            ins=[
                eng.lower_ap(in_ap),
                mybir.ImmediateValue(dtype=F32, value=0.0),
                mybir.ImmediateValue(dtype=F32, value=1.0),
                mybir.ImmediateValue(dtype=F32, value=0.0),
            ],
            outs=[eng.lower_ap(out_ap)],
        )
    )


def build_kernel(ctx: ExitStack, tc: tile.TileContext, out_d, cost_d, spg_d):
    nc = tc.nc
    tt = nc.vector.tensor_tensor

    cost_hdw = cost_d.transpose([1, 0, 2])  # [128(h), 48(d), 240(w)] view
    spg_v = spg_d.rearrange("c (R dr) (k w) -> c R dr k w", dr=4, k=2)
    out_v = out_d.rearrange("(R dr) (k w) -> R dr k w", dr=4, k=2)

    # ---- persistent tiles --------------------------------------------------
    pers = ctx.enter_context(tc.tile_pool(name="pers", bufs=1))
    rv = []
    urep = []
    for s_ in range(3):
        rv_s = pers.tile([128, W + 2], F32, tag=f"rv{s_}")
        rv.append(rv_s)
        urep_s = pers.tile([128, 4 * (W + 2)], FP16, tag=f"urep{s_}")
        urep.append(urep_s)
    small = ctx.enter_context(tc.tile_pool(name="small", bufs=1))
    for s_ in range(3):
        nc.vector.memset(rv[s_][:], 0.0)

    # ---- program-lifetime fine pools (stack allocator: first = outermost) --
    raw_pool = ctx.enter_context(tc.tile_pool(name="raw", bufs=2))
    e_pool = ctx.enter_context(tc.tile_pool(name="epool", bufs=9))
    fin = ctx.enter_context(tc.tile_pool(name="fin", bufs=1))
    r0p = ctx.enter_context(tc.tile_pool(name="r0p", bufs=1))
    p_pool = ctx.enter_context(tc.tile_pool(name="ppool", bufs=1))
    out_pool = ctx.enter_context(tc.tile_pool(name="outp", bufs=1))

    SPG_ORDER = [(k, c) for k in range(2) for c in range(9)]
    raws = {}
    spg_issued = [0]

    def issue_spg():
        if spg_issued[0] >= len(SPG_ORDER):
            return
        k, c = SPG_ORDER[spg_issued[0]]
        spg_issued[0] += 1
        raw = raw_pool.tile([128, 4, KW], F32, tag="raw")
        nc.sync.dma_start(raw[:], spg_v[c, :, :, k, :])
        raws[(k, c)] = raw

    e_tiles = {}
    for k in range(2):
        for c in range(9):
            e_kc = e_pool.tile([128, 4, KW], FP16, tag="e")
            e_tiles[(k, c)] = e_kc

    def emit_exp(k, c):
        nc.scalar.activation(e_tiles[(k, c)][:], raws[(k, c)][:], ACT.Exp)
        issue_spg()

    # ---- topk scope --------------------------------------------------------
    topk_scope = ExitStack()
    tkp = topk_scope.enter_context(tc.tile_pool(name="tkp", bufs=1))
    mtp = topk_scope.enter_context(tc.tile_pool(name="mtp", bufs=2))

    costF = tkp.tile([128, D, W], F32)
    v8 = tkp.tile([128, W, 8], F32)
    eta = tkp.tile([128, D, W], FP16)
    iotaF = tkp.tile([128, D], FP16)
    nc.gpsimd.iota(iotaF[:], [[1, D]], base=0, channel_multiplier=0,
                   allow_small_or_imprecise_dtypes=True)

    # cost DMA by w chunks (sync queue, ahead of the spg stream)
    w0 = 0
    wchunks = []
    for nw in COST_CHUNKS:
        nc.sync.dma_start(costF[:, :, w0:w0 + nw], cost_hdw[:, :, w0:w0 + nw])
        wchunks.append((w0, nw))
        w0 += nw
    for _ in range(2):
        issue_spg()
    # spg exps for k0, then eta = exp(cost) in one full-width ACT pass
    for c in range(9):
        emit_exp(0, c)
    nc.scalar.activation(eta[:], costF[:], ACT.Exp)

    for w0, nw in wchunks:
        for j in range(nw):
            nc.vector.max(out=v8[:, w0 + j], in_=costF[:, :, w0 + j])

    m2c = small.tile([128, W], F32, tag="m2c")
    nc.vector.tensor_copy(m2c[:], v8[:, :, 1])
    bshape = [128, D, W]
    M = mtp.tile([128, D, W], FP16, tag="mt")
    tt(M[:], costF[:], m2c[:].unsqueeze(1).broadcast_to(bshape), op=OP.is_ge)
    etaM = mtp.tile([128, D, W], FP16, tag="mt")
    tt(etaM[:], eta[:], M[:], op=OP.mult)

    def sum_ladder(dst, src3d):
        t_full = mtp.tile([128, D, W], FP16, tag="mt")
        t = t_full[:, 0:24, :]
        tt(t[:], src3d[:, 0:24, :], src3d[:, 24:48, :], op=OP.add)
        tt(t[:, 0:12, :], t[:, 0:12, :], t[:, 12:24, :], op=OP.add)
        tt(t[:, 0:6, :], t[:, 0:6, :], t[:, 6:12, :], op=OP.add)
        tt(t[:, 0:3, :], t[:, 0:3, :], t[:, 3:6, :], op=OP.add)
        tt(t[:, 0:1, :], t[:, 0:1, :], t[:, 1:2, :], op=OP.add)
        tt(dst[:].unsqueeze(1), t[:, 0:1, :], t[:, 2:3, :], op=OP.add)

    den2 = small.tile([128, W], F32, tag="den2")
    sum_ladder(den2, etaM[:])
    tt(eta[:], etaM[:], iotaF[:].unsqueeze(2).broadcast_to(bshape), op=OP.mult)
    num2 = small.tile([128, W], F32, tag="num2")
    sum_ladder(num2, eta[:])
    nc.vector.reciprocal(den2[:], den2[:])
    nc.vector.tensor_mul(rv[1][:, 1:1 + W], num2[:], den2[:])

    topk_scope.close()

    # ---- rv row shifts + x4 col repeat (x4 disp scale folded in) ----------
    nc.sync.dma_start(rv[0][1:128, 1:W + 2], rv[1][0:127, 1:W + 2])
    nc.sync.dma_start(rv[2][0:127, 1:W + 2], rv[1][1:128, 1:W + 2])
    for s_ in range(3):
        nc.scalar.activation(
            urep[s_][:].rearrange("p (x dw) -> p x dw", dw=4),
            rv[s_][:].unsqueeze(2).broadcast_to([128, W + 2, 4]),
            ACT.Copy, scale=4.0)

    # ---- fine stage --------------------------------------------------------
    num_t, den_t = {}, {}
    for k in range(2):
        num_k = fin.tile([128, 4, KW], FP16, tag=f"num{k}")
        num_t[k] = num_k
        den_k = fin.tile([128, 4, KW], FP16, tag=f"den{k}")
        den_t[k] = den_k

    def u4(k, c):
        ci, cj = c // 3, c % 3
        off = 4 * (k * (W // 2) + cj)
        return (urep[ci][:, off:off + KW]
                .unsqueeze(1).broadcast_to([128, 4, KW]))

    def fine_channel(k, c):
        e = e_tiles[(k, c)]
        num = num_t[k]
        den = den_t[k]
        if c == 0:
            nc.vector.tensor_mul(num[:], e[:], u4(k, 0))
        else:
            p = p_pool.tile([128, 4, KW], FP16, tag="p")
            nc.vector.tensor_mul(p[:], e[:], u4(k, c))
            nc.vector.tensor_add(num[:], num[:], p[:])
        # den: c1..c4 on gpsimd (contiguous adds), c5..c8 on DVE (tail)
        if c == 1:
            nc.gpsimd.tensor_add(den[:], e_tiles[(k, 0)][:], e[:])
        elif 2 <= c <= 4:
            nc.gpsimd.tensor_add(den[:], den[:], e[:])
        elif c >= 5:
            nc.vector.tensor_add(den[:], den[:], e[:])

    def fine_final(k):
        r0 = r0p.tile([128, 4, KW], FP16, tag="r0")
        _act_reciprocal(nc, r0[:], den_t[k][:])
        outt = out_pool.tile([128, 4, KW], FP16, tag="outt")
        nc.vector.tensor_mul(outt[:], num_t[k][:], r0[:])
        nc.sync.dma_start(out_v[:, :, k, :], outt[:])

    for c in range(9):
        fine_channel(0, c)
    fine_final(0)
    for c in range(9):
        emit_exp(1, c)
        fine_channel(1, c)
    fine_final(1)


def build_program():
    nc = bacc.Bacc(
        "TRN2",
        target_bir_lowering=False,
        debug=False,
        enable_asserts=False,
        num_devices=N_CORES,
    )
    cost_d = nc.dram_tensor("cost", [D, H, W], F32, kind="ExternalInput").ap()
    spg_d = nc.dram_tensor("spg", [9, HF, WF], F32, kind="ExternalInput").ap()
    out_d = nc.dram_tensor("out", [HF, WF], FP16, kind="ExternalOutput").ap()
    with tile.TileContext(nc) as tc:
        with ExitStack() as ctx:
            build_kernel(ctx, tc, out_d, cost_d, spg_d)
    nc.compile()
    return nc


def _install_ntff_hook():
    """Provide antenv.axon_hooks + register the ctypes NTFF profiler."""
    import types

    if "antenv.axon_hooks" in sys.modules:
        return True
    try:
        import antenv
        from trn_agent_boot.trn_boot import _ntff_profile_via_ctypes

        mod = types.ModuleType("antenv.axon_hooks")
        mod._hook = None

        def set_axon_ntff_profile_hook(hook):
            mod._hook = hook

        def get_axon_ntff_profile_hook():
            return mod._hook

        mod.set_axon_ntff_profile_hook = set_axon_ntff_profile_hook
        mod.get_axon_ntff_profile_hook = get_axon_ntff_profile_hook
        sys.modules["antenv.axon_hooks"] = mod
        antenv.axon_hooks = mod
        mod._hook = _ntff_profile_via_ctypes("/opt/axon/libaxon_pjrt.so")
        return True
    except Exception as e:  # profiling is best-effort
        print(f"NTFF hook install failed: {e}")
        return False


LAST_RESULTS = None


def kernel(cost: np.ndarray, spg: np.ndarray) -> np.ndarray:
    """cost [8,1,48,128,240] f32, spg [8,9,512,960] f32 -> disp1 [8,512,960] f32."""
    global LAST_RESULTS
    cost = np.ascontiguousarray(np.asarray(cost, dtype=np.float32))
    spg = np.ascontiguousarray(np.asarray(spg, dtype=np.float32))
    assert cost.shape == (B, 1, D, H, W) and spg.shape == (B, 9, HF, WF)

    nc = build_program()
    in_maps = [
        {"cost": cost[b, 0], "spg": spg[b]} for b in range(B)
    ]
    trace = bool(int(os.environ.get("KERNEL_TRACE", "0")))
    if trace:
        trace = _install_ntff_hook()
    res = run_bass_kernel_spmd(
        nc, in_maps, core_ids=list(range(N_CORES)), trace=trace
    )
    LAST_RESULTS = res
    out = np.stack([np.asarray(res.results[b]["out"]) for b in range(B)], axis=0)
    return out.astype(np.float32)
